# revision 1
# baseline (speedup 1.0000x reference)
"""Multi-head attention (dense_transformer) Trainium2 Bass kernel.

Problem: x[8, 512, 32, 32]; per-batch 1x1-conv QKV projections, 8-head
attention over N=H*W=1024 positions (head_dim 64), output projection,
residual. Sharding: data-parallel over batch B=8 across the 8 cores —
one batch element per core, no collectives.

Per-core dataflow (all matmul inputs bf16, accumulation fp32):
  - Host pre-transposes weights to [c, o] layout and pre-casts to bf16.
  - Q, K in [c, n] layout: Q[ot] = WqT[ct].T @ x16[ct] (+bq).
  - V kept transposed: VT[jt][n, o] = x16[:, jt].T @ WvT (+bv), stored
    per-head with a ones column appended: [128, 8 heads, 65].
  - S^T[j, i] = K_h.T Q_h per head: j on partitions -> AV matmul needs
    no transposes anywhere. exp via ScalarE with the 1/sqrt(64) scale
    folded in; softmax denominator comes from the VT ones column during
    the AV matmul (PSUM row 64); normalization = reciprocal + DRAM-
    bounce partition broadcast + VectorE multiply.
  - out = WoT.T @ O + (x32 + bo prefolded), DMA'd out in fp32.

PSUM (8 banks) is phase-scoped: projections use a 4-buf half-bank pool
that closes before the AV-accumulator pool opens in the same banks.
"""

import sys

if "/opt/trn_rl_repo" not in sys.path:
    sys.path.insert(0, "/opt/trn_rl_repo")

import numpy as np
import ml_dtypes

import concourse.bass as bass
import concourse.mybir as mybir
from concourse.tile import TileContext

DIM = 512
NH = 8
HD = 64
N = 1024
P = 128
CT = DIM // P  # 4 c-tiles of 128 channels
JT = N // P    # 8 j-tiles of 128 positions
F32 = mybir.dt.float32
BF16 = mybir.dt.bfloat16
AOP = mybir.AluOpType
EXP = mybir.ActivationFunctionType.Exp


class FixedTileContext(TileContext):
    """Works around a walrus/bass snapshot mismatch: this walrus build
    accepts only one sync-wait command per instruction, but Tile's wait
    assigner happily attaches several. After scheduling, excess waits on
    any instruction are peeled off onto same-engine NOPs inserted right
    before it (same blocking semantics: the engine executes in order)."""

    MAX_WAITS = 1
    MAX_WAITS_DATA = 1
    _wsplit_ctr = 0

    def _split_sync_waits(self):
        seq_only = mybir.SEQUENCER_ONLY_OPCODES
        for fn in self.nc.m.functions:
            for blk in fn.blocks:
                insts = list(blk.instructions)
                out = []
                for inst in insts:
                    si = inst.sync_info
                    limit = (
                        self.MAX_WAITS
                        if inst.opcode in seq_only
                        else self.MAX_WAITS_DATA
                    )
                    if si is not None and len(si.on_wait) > limit:
                        waits = list(si.on_wait)
                        movers = waits[:-limit]
                        keep = waits[-limit:]
                        del si.on_wait[:]
                        for w in keep:
                            si.on_wait.append(w)
                        for w in movers:
                            FixedTileContext._wsplit_ctr += 1
                            nop = mybir.InstNoOp(
                                name=f"wsplit-{FixedTileContext._wsplit_ctr}",
                                ins=[],
                                outs=[],
                            )
                            nop.engine = inst.engine
                            nop.sync_info = mybir.SyncInfo(on_wait=[w], on_update=[])
                            out.append(nop)
                    out.append(inst)
                if len(out) != len(insts):
                    del blk.instructions[:]
                    for i in out:
                        blk.add_instruction(i)

    split_on_exit = True

    def __exit__(self, *exc):
        ret = super().__exit__(*exc)
        if exc[0] is None and self.split_on_exit:
            self._split_sync_waits()
        return ret


def build_nc(split_waits=True):
    nc = bass.Bass()

    x32d = nc.dram_tensor("x32", [DIM, N], F32, kind="ExternalInput")
    x16d = nc.dram_tensor("x16", [DIM, N], BF16, kind="ExternalInput")
    wqd = nc.dram_tensor("wqt", [DIM, DIM], BF16, kind="ExternalInput")
    wkd = nc.dram_tensor("wkt", [DIM, DIM], BF16, kind="ExternalInput")
    wvd = nc.dram_tensor("wvt", [DIM, DIM], BF16, kind="ExternalInput")
    wod = nc.dram_tensor("wot", [DIM, DIM], BF16, kind="ExternalInput")
    bqd = nc.dram_tensor("bq", [DIM], F32, kind="ExternalInput")
    bkd = nc.dram_tensor("bk", [DIM], F32, kind="ExternalInput")
    bvd = nc.dram_tensor("bv", [DIM], F32, kind="ExternalInput")
    bod = nc.dram_tensor("bo", [DIM], F32, kind="ExternalInput")
    outd = nc.dram_tensor("out", [DIM, N], F32, kind="ExternalOutput")

    FixedTileContext.split_on_exit = split_waits
    with FixedTileContext(nc) as tc:
        with (
            tc.tile_pool(name="persist", bufs=1) as persist,
            tc.tile_pool(name="ppool", bufs=32) as ppool,
            tc.tile_pool(name="small", bufs=3) as small,
            tc.tile_pool(name="otile", bufs=8) as otile,
            tc.tile_pool(name="dram", bufs=1, space="DRAM") as dram,
            tc.tile_pool(name="psS", bufs=2, space="PSUM") as psS_pool,
        ):
            # weights/biases ride ScalarE's DMA queues (ScalarE is idle
            # until the first exp) so they don't serialize behind the x
            # loads on SP's queues
            def load_w(wd, name):
                wr = wd.rearrange("(t p) o -> t p o", p=P)
                ws = []
                for t in range(CT):
                    wt = persist.tile(
                        [P, DIM], BF16, tag=f"{name}_{t}", name=f"{name}_{t}"
                    )
                    nc.scalar.dma_start(out=wt, in_=wr[t])
                    ws.append(wt)
                return ws

            def load_b(bd, name):
                bt = persist.tile([P, CT], F32, tag=name, name=name)
                nc.scalar.dma_start(out=bt, in_=bd.rearrange("(t p) -> p t", p=P))
                return bt

            # S^T + exp for one head pair. Emission alternates PE row
            # groups 0-63 / 64-127 between consecutive matmuls so the
            # hardware overlaps them (per-subarray concurrency) even
            # though K=64 only half-fills the array.
            def s_phase(pair):
                P16 = {}
                for jt in range(JT):
                    tiles = {}

                    def smm(h2, ih):
                        base = 64 * h2
                        nc.tensor.matmul(
                            tiles[h2][:, ih * 512 : (ih + 1) * 512],
                            lhsT=K[pair][base : base + 64, jt * P : (jt + 1) * P],
                            rhs=Q[pair][base : base + 64, ih * 512 : (ih + 1) * 512],
                            start=True,
                            stop=True,
                        )

                    tiles[0] = psS_pool.tile([P, N], F32, tag="psS", name="psS")
                    smm(0, 0)
                    tiles[1] = psS_pool.tile([P, N], F32, tag="psS", name="psS")
                    smm(1, 0)
                    smm(0, 1)
                    smm(1, 1)
                    for h2 in range(2):
                        pt = ppool.tile([P, N], BF16, tag="p16", name="p16")
                        nc.scalar.activation(pt, tiles[h2], EXP, scale=0.125)
                        P16[(jt, h2)] = pt
                return P16

            def p16_slice(P16, jt, h2, ih):
                return P16[(jt, h2)][:, ih * 512 : (ih + 1) * 512]

            # AV matmul + softmax normalization for one head pair. The raw
            # head output is copied out of PSUM right away (frees the psO
            # slot for the next head's AV); the DRAM-bounce broadcast and
            # the normalize multiply then run off the critical PSUM path.
            def av_phase(pair, P16, psO_pool, O16, rdram):
                last_pair = pair == NH // 2 - 1
                h2_order = (1, 0) if last_pair else (0, 1)
                for h2 in h2_order:
                    h = 2 * pair + h2
                    rec = small.tile([HD + 1, N], F32, tag="rec", name="rec")
                    oraw = small.tile([HD, N], F32, tag="oraw", name="oraw")
                    rb = small.tile([HD, N], F32, tag="rb", name="rb")
                    for ih in range(2):
                        sl = slice(ih * 512, (ih + 1) * 512)
                        po = psO_pool.tile([HD + 1, 512], F32, tag="psO", name="po")
                        for jt in range(JT):
                            nc.tensor.matmul(
                                po,
                                lhsT=VT[jt][:, h, :],
                                rhs=p16_slice(P16, jt, h2, ih),
                                start=(jt == 0),
                                stop=(jt == JT - 1),
                            )
                        # softmax denominator sits in row HD of po
                        nc.vector.reciprocal(rec[HD : HD + 1, sl], po[HD : HD + 1, :])
                        # copy the raw head output out of PSUM immediately
                        # (frees the psO slot); on the last pair ScalarE is
                        # done with exps, so use it and keep DVE off the
                        # critical chain
                        if last_pair:
                            nc.scalar.copy(oraw[:, sl], po[0:HD, :])
                        else:
                            nc.vector.tensor_copy(oraw[:, sl], po[0:HD, :])
                        # per-half DRAM bounce broadcasts 1/colsum across
                        # partitions (SBUF APs reject 0 partition stride)
                        dmae = nc.scalar if last_pair else nc.sync
                        dmae.dma_start(
                            out=rdram[h : h + 1, sl], in_=rec[HD : HD + 1, sl]
                        )
                        rsrc = rdram[h : h + 1, sl]
                        nc.sync.dma_start(
                            out=rb[:, sl],
                            in_=bass.AP(
                                tensor=rsrc.tensor,
                                offset=rsrc.offset,
                                ap=[[0, HD]] + list(rsrc.ap[1:]),
                            ),
                        )
                    osc = None
                    if h2 != 0:
                        osc = small.tile([HD, N], BF16, tag="osc", name="osc")
                    for ih in range(2):
                        sl = slice(ih * 512, (ih + 1) * 512)
                        if h2 == 0:
                            nc.vector.tensor_tensor(
                                O16[pair][0:HD, sl], oraw[:, sl], rb[:, sl], AOP.mult
                            )
                        else:
                            nc.vector.tensor_tensor(
                                osc[:, sl], oraw[:, sl], rb[:, sl], AOP.mult
                            )
                            (nc.scalar if last_pair else nc.sync).dma_start(
                                out=O16[pair][HD:P, sl], in_=osc[:, sl]
                            )

            with tc.tile_pool(name="pp", bufs=4, space="PSUM") as pp:
                # ---------- input loads ----------
                x16r = x16d.rearrange("(t p) n -> t p n", p=P)
                xs16 = []
                for t in range(CT):
                    xt = persist.tile([P, N], BF16, tag=f"x16_{t}", name=f"x16_{t}")
                    nc.sync.dma_start(out=xt, in_=x16r[t])
                    xs16.append(xt)

                # interleave wq/wk tiles so K0's accumulation matmuls can
                # trickle-start alongside Q0's instead of waiting for the
                # whole of wq to finish on the same queue
                wqr = wqd.rearrange("(t p) o -> t p o", p=P)
                wkr = wkd.rearrange("(t p) o -> t p o", p=P)
                wqs, wks = [], []
                for t in range(CT):
                    wqt_ = persist.tile([P, DIM], BF16, tag=f"wq_{t}", name=f"wq_{t}")
                    nc.scalar.dma_start(out=wqt_, in_=wqr[t])
                    wqs.append(wqt_)
                    wkt_ = persist.tile([P, DIM], BF16, tag=f"wk_{t}", name=f"wk_{t}")
                    nc.scalar.dma_start(out=wkt_, in_=wkr[t])
                    wks.append(wkt_)
                bq_sb = load_b(bqd, "bq")
                bk_sb = load_b(bkd, "bk")

                # trigger the ~2.7us exp table load on ScalarE right after
                # its weight-DMA issues, so the first real exp doesn't pay it
                warm = small.tile([1, 8], F32, tag="warm", name="warm")
                nc.vector.memset(warm, 0.0)
                nc.scalar.activation(warm, warm, EXP)

                # ------ Q, K projections: [CT][128, N] bf16, [c, n] layout
                def project_one(ws, b_sb, name, ot):
                    qt = persist.tile(
                        [P, N], BF16, tag=f"{name}_{ot}", name=f"{name}_{ot}"
                    )
                    for nh in range(2):
                        ps = pp.tile(
                            [P, 512], F32, tag="pp", name=f"pp_{name}{ot}{nh}"
                        )
                        for ct in range(CT):
                            nc.tensor.matmul(
                                ps,
                                lhsT=ws[ct][:, ot * P : (ot + 1) * P],
                                rhs=xs16[ct][:, nh * 512 : (nh + 1) * 512],
                                start=(ct == 0),
                                stop=(ct == CT - 1),
                            )
                        nc.vector.tensor_scalar_add(
                            qt[:, nh * 512 : (nh + 1) * 512],
                            ps,
                            b_sb[:, ot : ot + 1],
                        )
                    return qt

                Q, K = [], []
                Q.append(project_one(wqs, bq_sb, "q", 0))
                K.append(project_one(wks, bk_sb, "k", 0))

                # pair 0's S^T + exp right away: gets ScalarE going while
                # the remaining projections stream on the PE
                P16_0 = s_phase(0)
                Q.append(project_one(wqs, bq_sb, "q", 1))
                K.append(project_one(wks, bk_sb, "k", 1))
                P16_1 = s_phase(1)

                # ------ V^T projection: VT[jt] = [128, NH, HD+1] bf16
                wvs = load_w(wvd, "wv")
                bvB = persist.tile([P, DIM], F32, tag="bvB", name="bvB")
                nc.gpsimd.dma_start(
                    out=bvB,
                    in_=bass.AP(
                        tensor=bvd[:].tensor, offset=0, ap=[[0, P], [1, DIM]]
                    ),
                )
                VT = []
                for jt in range(JT):
                    vt = persist.tile(
                        [P, NH, HD + 1], BF16, tag=f"vt_{jt}", name=f"vt_{jt}"
                    )
                    ps = pp.tile([P, 512], F32, tag="pp", name=f"pp_v{jt}")
                    for ct in range(CT):
                        nc.tensor.matmul(
                            ps,
                            lhsT=xs16[ct][:, jt * P : (jt + 1) * P],
                            rhs=wvs[ct],
                            start=(ct == 0),
                            stop=(ct == CT - 1),
                        )
                    nc.vector.tensor_tensor(
                        vt[:, :, 0:HD],
                        ps.rearrange("p (h d) -> p h d", h=NH),
                        bvB.rearrange("p (h d) -> p h d", h=NH),
                        AOP.add,
                    )
                    nc.vector.memset(vt[:, :, HD : HD + 1], 1.0)
                    VT.append(vt)

                for ot in range(2, CT):
                    Q.append(project_one(wqs, bq_sb, "q", ot))
                    K.append(project_one(wks, bk_sb, "k", ot))

            # ---------- attention (heads 2p / 2p+1 live on partitions
            # 0-63 / 64-127 of Q/K c-tile p); the AV-accumulator pool
            # reuses banks the projection pool just released
            O16 = [
                persist.tile([P, N], BF16, tag=f"o16_{t}", name=f"o16_{t}")
                for t in range(CT)
            ]
            rdram = dram.tile([NH, N], F32, tag="rdram", name="rdram")
            with tc.tile_pool(name="psO", bufs=4, space="PSUM") as psO_pool:
                av_phase(0, P16_0, psO_pool, O16, rdram)
                P16_2 = s_phase(2)
                av_phase(1, P16_1, psO_pool, O16, rdram)
                P16_3 = s_phase(3)
                av_phase(2, P16_2, psO_pool, O16, rdram)
                av_phase(3, P16_3, psO_pool, O16, rdram)

                # loads for the output projection (low priority; the DMA
                # queues have slack mid-kernel)
                wos = load_w(wod, "wo")
                bo_sb = load_b(bod, "bo")
                x32r = x32d.rearrange("(t p) n -> t p n", p=P)
                xs32 = []
                for t in range(CT):
                    xt32 = persist.tile(
                        [P, N], F32, tag=f"x32_{t}", name=f"x32_{t}"
                    )
                    nc.sync.dma_start(out=xt32, in_=x32r[t])
                    nc.vector.tensor_scalar_add(xt32, xt32, bo_sb[:, t : t + 1])
                    xs32.append(xt32)

            # ---------- output projection + residual. ot0/ot1 psum tiles
            # come from the psS pool (slots drained by pair-3 exps);
            # ot2/ot3 from a pool reusing the psO banks (drained by the
            # early PSUM copies) — all 24 ct0-2 matmuls can therefore run
            # while the last head's epilogue is still in flight.
            with tc.tile_pool(name="po3", bufs=2, space="PSUM") as po3:
                outr = outd.rearrange("(t p) n -> t p n", p=P)

                def op_pre(ot, pool=None):
                    # ct 0..2 accumulation: issuable while the last head
                    # pair (feeding O16[3]) is still in its epilogue
                    if pool is None:
                        ps = psS_pool.tile([P, N], F32, tag="psS", name=f"ps_o{ot}")
                    else:
                        ps = pool.tile([P, N], F32, tag="op34", name=f"ps_o{ot}")
                    for nh in range(2):
                        for ct in range(CT - 1):
                            nc.tensor.matmul(
                                ps[:, nh * 512 : (nh + 1) * 512],
                                lhsT=wos[ct][:, ot * P : (ot + 1) * P],
                                rhs=O16[ct][:, nh * 512 : (nh + 1) * 512],
                                start=(ct == 0),
                                stop=(ct == CT - 2),
                            )
                    return ps

                def op_post(ot, ps):
                    # ct 3 continues the accumulation in a second group,
                    # then bias+residual and writeback
                    for nh in range(2):
                        nc.tensor.matmul(
                            ps[:, nh * 512 : (nh + 1) * 512],
                            lhsT=wos[CT - 1][:, ot * P : (ot + 1) * P],
                            rhs=O16[CT - 1][:, nh * 512 : (nh + 1) * 512],
                            start=False,
                            stop=True,
                            skip_group_check=True,
                        )
                    for nh in range(2):
                        ob = otile.tile([P, 512], F32, tag="ob", name="ob")
                        nc.vector.tensor_tensor(
                            ob,
                            ps[:, nh * 512 : (nh + 1) * 512],
                            xs32[ot][:, nh * 512 : (nh + 1) * 512],
                            AOP.add,
                        )
                        nc.sync.dma_start(
                            out=outr[ot][:, nh * 512 : (nh + 1) * 512], in_=ob
                        )

                ps0 = op_pre(0)
                ps1 = op_pre(1)
                ps2 = op_pre(2, po3)
                ps3 = op_pre(3, po3)
                op_post(0, ps0)
                op_post(1, ps1)
                op_post(2, ps2)
                op_post(3, ps3)
    return nc


_BF = ml_dtypes.bfloat16


def _prep_maps(x, Wq, bq, Wk, bk, Wv, bv, Wo, bo):
    # plain numpy up front: inputs may arrive as jax device arrays and
    # transforming those would trigger on-device jax execution
    x, Wq, bq, Wk, bk, Wv, bv, Wo, bo = (
        np.asarray(a) for a in (x, Wq, bq, Wk, bk, Wv, bv, Wo, bo)
    )
    B, C, H, W = x.shape
    xf = np.ascontiguousarray(x.reshape(B, C, H * W)).astype(np.float32)
    shared = {
        "wqt": np.ascontiguousarray(Wq.T).astype(_BF),
        "wkt": np.ascontiguousarray(Wk.T).astype(_BF),
        "wvt": np.ascontiguousarray(Wv.T).astype(_BF),
        "wot": np.ascontiguousarray(Wo.T).astype(_BF),
        "bq": np.asarray(bq, np.float32),
        "bk": np.asarray(bk, np.float32),
        "bv": np.asarray(bv, np.float32),
        "bo": np.asarray(bo, np.float32),
    }
    in_maps = []
    for b in range(B):
        m = dict(shared)
        m["x32"] = xf[b]
        m["x16"] = xf[b].astype(_BF)
        in_maps.append(m)
    return in_maps


def kernel(x, Wq, bq, Wk, bk, Wv, bv, Wo, bo, _trace=False):
    from concourse.bass_utils import run_bass_kernel_spmd

    x = np.asarray(x)
    B, C, H, W = x.shape
    in_maps = _prep_maps(x, Wq, bq, Wk, bk, Wv, bv, Wo, bo)
    nc = build_nc()
    res = run_bass_kernel_spmd(nc, in_maps, core_ids=list(range(B)), trace=_trace)
    out = np.stack([res.results[b]["out"] for b in range(B)])
    out = out.reshape(B, C, H, W).astype(np.float32)
    if _trace:
        kernel.last_results = res
    return out



# revision 8
# speedup vs baseline: 1.0556x; 1.0556x over previous
"""Multi-head attention (dense_transformer) Trainium2 Bass kernel, v2.

Problem: x[8, 512, 32, 32]; per-batch 1x1-conv QKV projections, 8-head
attention over N=H*W=1024 positions (head_dim 64), output projection,
residual. Sharding: data-parallel over batch B=8 across the 8 cores -
one batch element per core, no collectives.

v2 strategy (vs the bf16 baseline): run every matmul in fp8e4 with the
DoubleRow perf mode.  A DoubleRow matmul takes lhsT [K,2,M] / rhs
[K,2,N] and computes sum_g W[:,g].T @ X[:,g] - two 128-deep contraction
steps in one instruction at 0.5 cycles per output row, i.e. 4x the
bf16 matmul throughput per unit of contracted work.  That collapses the
PE time from ~82us to ~30us and shifts the bottleneck to the softmax
exp (~65k free-elems), which is split across three engines:

  - ScalarE (Act): true exp via the activation LUT.
  - VectorE / GpSimd: Schraudolph-style fast exp - one fused
    tensor_scalar (x*log2e + bias) written as int8 IS the fp8e4 bit
    pattern of exp(x/8).  (e4m3 bits b ~= 8*(log2(v)+7), so
    b = s*log2(e) + 56 up to a mantissa correction.)

Other tricks:
  - All five "c = 512" contractions map c -> (ctpair, group, partition)
    so DoubleRow pairs come straight from SBUF layout; Wq/Wk columns are
    host-permuted so each head's Q/K lands as [32, 2, N] (d-halves in
    the group dim) on its own 32-partition band, making S^T a single
    DoubleRow matmul per output tile.
  - V is projected transposed (VT[j, head, d]) with stationary columns
    64..127 set to 1.0, so the AV matmul leaves rows 64..127 of PSUM
    holding the softmax denominator replicated 64x - normalization is
    one partition-offset tensor_tensor divide, no broadcasts.
  - Output projection accumulates Wo.T O (fp8 DoubleRow), + bo (K=1
    DoubleRow bias matmul), + x (bf16 identity matmul residual) in one
    PSUM group; the only epilogue is a PSUM->SBUF copy before DMA out.
"""

import sys

if "/opt/trn_rl_repo" not in sys.path:
    sys.path.insert(0, "/opt/trn_rl_repo")

import numpy as np
import ml_dtypes

import concourse.bass as bass
import concourse.mybir as mybir
from concourse.tile import TileContext

DIM = 512
NH = 8
HD = 64
N = 1024
P = 128
F32 = mybir.dt.float32
BF16 = mybir.dt.bfloat16
F8 = mybir.dt.float8e4
I8 = mybir.dt.int8
AOP = mybir.AluOpType
EXP = mybir.ActivationFunctionType.Exp
DR = mybir.MatmulPerfMode.DoubleRow

# Schraudolph fast-exp constants for e4m3 bits: for logits s (pre 1/8
# scale), bits = s*log2(e) + 7*8 - 0.344 (mantissa-linearity centering)
# + 0.5 (CoreSim truncates on f32->i8 write; HW rounding only shifts
# the softmax by a uniform factor that the normalization cancels).
EXP_C1 = 1.4426950408889634
EXP_C2 = 56.0 - 0.344 + 0.5


class FixedTileContext(TileContext):
    """Works around a walrus/bass snapshot mismatch: this walrus build
    accepts only one sync-wait command per instruction, but Tile's wait
    assigner happily attaches several. After scheduling, excess waits on
    any instruction are peeled off onto same-engine NOPs inserted right
    before it (same blocking semantics: the engine executes in order)."""

    MAX_WAITS = 1
    MAX_WAITS_DATA = 1
    _wsplit_ctr = 0

    def _split_sync_waits(self):
        seq_only = mybir.SEQUENCER_ONLY_OPCODES
        for fn in self.nc.m.functions:
            for blk in fn.blocks:
                insts = list(blk.instructions)
                out = []
                for inst in insts:
                    si = inst.sync_info
                    limit = (
                        self.MAX_WAITS
                        if inst.opcode in seq_only
                        else self.MAX_WAITS_DATA
                    )
                    if si is not None and len(si.on_wait) > limit:
                        waits = list(si.on_wait)
                        movers = waits[:-limit]
                        keep = waits[-limit:]
                        del si.on_wait[:]
                        for w in keep:
                            si.on_wait.append(w)
                        for w in movers:
                            FixedTileContext._wsplit_ctr += 1
                            nop = mybir.InstNoOp(
                                name=f"wsplit-{FixedTileContext._wsplit_ctr}",
                                ins=[],
                                outs=[],
                            )
                            nop.engine = inst.engine
                            nop.sync_info = mybir.SyncInfo(on_wait=[w], on_update=[])
                            out.append(nop)
                    out.append(inst)
                if len(out) != len(insts):
                    del blk.instructions[:]
                    for i in out:
                        blk.add_instruction(i)

    split_on_exit = True

    def __exit__(self, *exc):
        ret = super().__exit__(*exc)
        if exc[0] is None and self.split_on_exit:
            self._split_sync_waits()
        return ret


def build_nc(split_waits=True):
    nc = bass.Bass()

    x8d = nc.dram_tensor("x8", [2, P, 2, N], F8, kind="ExternalInput")
    x16d = nc.dram_tensor("x16", [4, P, N], BF16, kind="ExternalInput")
    wqkd = nc.dram_tensor("wqk8", [2, P, 2, 2, DIM], F8, kind="ExternalInput")
    wvod = nc.dram_tensor("wvo8", [2, P, 2, 2, DIM], F8, kind="ExternalInput")
    bqkd = nc.dram_tensor("bqk", [P, 8], F32, kind="ExternalInput")
    bvd = nc.dram_tensor("bv", [DIM], F32, kind="ExternalInput")
    bo8d = nc.dram_tensor("bo8", [1, 2, DIM], F8, kind="ExternalInput")
    onesd = nc.dram_tensor("ones8", [N], F8, kind="ExternalInput")
    identd = nc.dram_tensor("ident16", [P, P], BF16, kind="ExternalInput")
    outd = nc.dram_tensor("out", [4, P, N], F32, kind="ExternalOutput")

    FixedTileContext.split_on_exit = split_waits
    with FixedTileContext(nc) as tc:
        with tc.tile_pool(name="persist", bufs=1) as persist, tc.tile_pool(
            name="otile", bufs=2
        ) as otile:
            # ---------------- loads ----------------
            # SP queue: x8 + wqk first (gate the first matmul), then
            # VT-ones / x16 / phase-3 constants (needed much later).
            x8sb = []
            for cp in range(2):
                t = persist.tile([P, 2, N], F8, tag=f"x8_{cp}", name=f"x8_{cp}")
                nc.sync.dma_start(out=t, in_=x8d[cp])
                x8sb.append(t)
            wqk = []
            for cp in range(2):
                t = persist.tile(
                    [P, 2, 2, DIM], F8, tag=f"wqk_{cp}", name=f"wqk_{cp}"
                )
                nc.sync.dma_start(out=t, in_=wqkd[cp])
                wqk.append(t)
            # Act queue: wvo + small f32 constants (Act's first consumer,
            # the Q epilogue, is ~3us in; 4 DMA configs cost ~2.7us of
            # Act SEQ which pipelines ahead of the engine).
            wvo = []
            for cp in range(2):
                t = persist.tile(
                    [P, 2, 2, DIM], F8, tag=f"wvo_{cp}", name=f"wvo_{cp}"
                )
                nc.scalar.dma_start(out=t, in_=wvod[cp])
                wvo.append(t)
            bqk_sb = persist.tile([P, 8], F32, tag="bqk", name="bqk")
            nc.scalar.dma_start(out=bqk_sb, in_=bqkd[:, :])
            # bv broadcast across partitions and the head-group dim via
            # zero strides on the DRAM side.
            bvB = persist.tile([P, 2, NH, HD], F32, tag="bvB", name="bvB")
            bv_ap = bvd[:]
            nc.scalar.dma_start(
                out=bvB,
                in_=bass.AP(
                    tensor=bv_ap.tensor,
                    offset=0,
                    ap=[[0, P], [0, 2], [1, DIM]],
                ),
            )

            # VT tiles: [j-partition, jt-group, head, 128] where columns
            # 0..63 are V (+bv) and 64..127 are 1.0 (denominator trick).
            VT = [
                persist.tile([P, 2, NH, P], F8, tag=f"vt_{jp}", name=f"vt_{jp}")
                for jp in range(4)
            ]
            ones_ap = onesd[:]
            for jp in range(4):
                nc.sync.dma_start(
                    out=VT[jp][:, :, :, HD:P],
                    in_=bass.AP(
                        tensor=ones_ap.tensor, offset=0, ap=[[0, P], [1, N]]
                    ),
                )
            # phase-3 inputs (SP queue keeps filling while PE works)
            x16sb = []
            for ot in range(4):
                t = persist.tile([P, N], BF16, tag=f"x16_{ot}", name=f"x16_{ot}")
                nc.sync.dma_start(out=t, in_=x16d[ot])
                x16sb.append(t)
            ident16 = persist.tile([P, P], BF16, tag="ident", name="ident")
            nc.sync.dma_start(out=ident16, in_=identd[:, :])
            bo8sb = persist.tile([1, 2, DIM], F8, tag="bo8", name="bo8")
            nc.sync.dma_start(out=bo8sb, in_=bo8d[:, :, :])
            ones8 = persist.tile([1, 2, DIM], F8, tag="ones8", name="ones8")
            nc.sync.dma_start(
                out=ones8,
                in_=bass.AP(tensor=ones_ap.tensor, offset=0, ap=[[0, 1], [1, N]]),
            )

            # persistent attention state
            Qs = [
                persist.tile([P, 2, N], F8, tag=f"qs_{i}", name=f"qs_{i}")
                for i in range(2)
            ]
            Ks = [
                persist.tile([P, 2, N], F8, tag=f"ks_{i}", name=f"ks_{i}")
                for i in range(2)
            ]
            P8 = [
                [
                    persist.tile(
                        [P, 2, N], F8, tag=f"p8_{h}_{jp}", name=f"p8_{h}_{jp}"
                    )
                    for jp in range(4)
                ]
                for h in range(NH)
            ]
            O8 = [
                persist.tile([P, 2, N], F8, tag=f"o8_{cp}", name=f"o8_{cp}")
                for cp in range(2)
            ]
            stage = [
                persist.tile([HD, N], F8, tag=f"stg_{i}", name=f"stg_{i}")
                for i in range(4)
            ]

            # ------------- Q/K/V projections -------------
            with tc.tile_pool(name="pp", bufs=4, space="PSUM") as pp:
                def qk_proj(qk, ot):
                    ps = pp.tile([P, N], F32, tag="pp", name=f"pp_{qk}{ot}")
                    for nh2 in range(2):
                        for cp in range(2):
                            nc.tensor.matmul(
                                ps[:, nh2 * 512 : (nh2 + 1) * 512],
                                lhsT=wqk[cp][:, :, qk, ot * P : (ot + 1) * P],
                                rhs=x8sb[cp][:, :, nh2 * 512 : (nh2 + 1) * 512],
                                start=(cp == 0),
                                stop=(cp == 1),
                                perf_mode=DR,
                            )
                    return ps

                for ot in range(4):
                    ps = qk_proj(0, ot)
                    # Q epilogue on Act: out = Identity(ps + bq_col), fp8
                    nc.scalar.activation(
                        Qs[ot // 2][:, ot % 2, :],
                        ps,
                        mybir.ActivationFunctionType.Identity,
                        bias=bqk_sb[:, ot : ot + 1],
                    )
                for ot in range(4):
                    ps = qk_proj(1, ot)
                    # K epilogue on GpSimd (tensor_scalar, default eff)
                    nc.gpsimd.tensor_scalar(
                        Ks[ot // 2][:, ot % 2, :],
                        ps,
                        bqk_sb[:, 4 + ot : 5 + ot],
                        None,
                        op0=AOP.add,
                    )
                for vt in range(4):
                    ps = pp.tile([P, N], F32, tag="pp", name=f"pp_v{vt}")
                    for g in range(2):
                        jt = 2 * vt + g
                        for cp in range(2):
                            nc.tensor.matmul(
                                ps[:, g * 512 : (g + 1) * 512],
                                lhsT=x8sb[cp][:, :, jt * P : (jt + 1) * P],
                                rhs=wvo[cp][:, :, 0, :],
                                start=(cp == 0),
                                stop=(cp == 1),
                                perf_mode=DR,
                            )
                    # V epilogue on DVE: V^T + bv -> fp8 into VT cols 0..63
                    nc.vector.tensor_tensor(
                        VT[vt][:, :, :, 0:HD],
                        ps.rearrange("p (g h d) -> p g h d", g=2, h=NH),
                        bvB,
                        AOP.add,
                    )

            # ------------- attention heads -------------
            # exp engine rotation per head: Act x4, DVE x2, GpSimd x2
            # (DVE also runs the normalization divide).
            rot = ["a", "d", "p", "a", "p", "a", "d", "a"]
            with tc.tile_pool(
                name="psS", bufs=2, space="PSUM"
            ) as psS_pool, tc.tile_pool(name="psO", bufs=2, space="PSUM") as psO_pool:
                for h in range(NH):
                    Qt = Qs[h // 4]
                    Kt = Ks[h // 4]
                    p0 = 32 * (h % 4)
                    for jt in range(8):
                        psS = psS_pool.tile([P, N], F32, tag="psS", name="psS")
                        for ih in range(2):
                            nc.tensor.matmul(
                                psS[:, ih * 512 : (ih + 1) * 512],
                                lhsT=Kt[p0 : p0 + 32, :, jt * P : (jt + 1) * P],
                                rhs=Qt[p0 : p0 + 32, :, ih * 512 : (ih + 1) * 512],
                                start=True,
                                stop=True,
                                perf_mode=DR,
                                tile_position=(p0, 0),
                            )
                        tgt = P8[h][jt // 2][:, jt % 2, :]
                        eng = rot[jt]
                        if eng == "a":
                            nc.scalar.activation(tgt, psS, EXP, scale=0.125)
                        elif eng == "d":
                            nc.vector.tensor_scalar(
                                tgt.bitcast(I8),
                                psS,
                                EXP_C1,
                                EXP_C2,
                                op0=AOP.mult,
                                op1=AOP.add,
                            )
                        else:
                            nc.gpsimd.tensor_scalar(
                                tgt.bitcast(I8),
                                psS,
                                EXP_C1,
                                EXP_C2,
                                op0=AOP.mult,
                                op1=AOP.add,
                            )
                    psO = psO_pool.tile([P, N], F32, tag="psO", name="psO")
                    for ih in range(2):
                        for jp in range(4):
                            nc.tensor.matmul(
                                psO[:, ih * 512 : (ih + 1) * 512],
                                lhsT=VT[jp][:, :, h, :],
                                rhs=P8[h][jp][:, :, ih * 512 : (ih + 1) * 512],
                                start=(jp == 0),
                                stop=(jp == 3),
                                perf_mode=DR,
                            )
                    # normalize: rows 64..127 all hold the denominator
                    cph, gh = h // 4, (h % 4) // 2
                    if h % 2 == 0:
                        nc.vector.tensor_tensor(
                            O8[cph][0:HD, gh, :],
                            psO[0:HD, :],
                            psO[HD:P, :],
                            AOP.divide,
                        )
                    else:
                        st = stage[h // 2]
                        nc.vector.tensor_tensor(
                            st, psO[0:HD, :], psO[HD:P, :], AOP.divide
                        )
                        nc.scalar.dma_start(out=O8[cph][HD:P, gh, :], in_=st)

            # ------------- output projection + residual -------------
            with tc.tile_pool(name="po", bufs=4, space="PSUM") as po_pool:
                for ot in range(4):
                    po = po_pool.tile([P, N], F32, tag="po", name=f"po_{ot}")
                    for nh2 in range(2):
                        sl = slice(nh2 * 512, (nh2 + 1) * 512)
                        for cp in range(2):
                            nc.tensor.matmul(
                                po[:, sl],
                                lhsT=wvo[cp][:, :, 1, ot * P : (ot + 1) * P],
                                rhs=O8[cp][:, :, sl],
                                start=(cp == 0),
                                stop=False,
                                perf_mode=DR,
                            )
                        nc.tensor.matmul(
                            po[:, sl],
                            lhsT=bo8sb[:, :, ot * P : (ot + 1) * P],
                            rhs=ones8[:, :, :],
                            start=False,
                            stop=False,
                            perf_mode=DR,
                        )
                        nc.tensor.matmul(
                            po[:, sl],
                            lhsT=ident16,
                            rhs=x16sb[ot][:, sl],
                            start=False,
                            stop=True,
                        )
                    ob = otile.tile([P, N], F32, tag="ob", name="ob")
                    if ot % 2 == 0:
                        nc.scalar.copy(ob, po)
                        nc.scalar.dma_start(out=outd[ot], in_=ob)
                    else:
                        nc.vector.tensor_copy(ob, po)
                        nc.sync.dma_start(out=outd[ot], in_=ob)
    return nc


_BF = ml_dtypes.bfloat16
_F8 = ml_dtypes.float8_e4m3


def _perm_qk():
    # PSUM partition p of Q/K projection tile `ot` holds output row
    # o = (4*(ot//2) + p//32)*64 + 32*(ot%2) + p%32  (head-banded,
    # d-halves split across the DoubleRow group dim).
    j = np.arange(DIM)
    ot, pp = j // P, j % P
    return (4 * (ot // 2) + pp // 32) * HD + 32 * (ot % 2) + (pp % 32)


def _fold8(a):
    # [512, M] c-major -> [ctpair, partition, group, M]
    M = a.shape[1]
    return np.ascontiguousarray(
        a.reshape(2, 2, P, M).transpose(0, 2, 1, 3)
    )


def _prep_maps(x, Wq, bq, Wk, bk, Wv, bv, Wo, bo):
    # plain numpy up front: inputs may arrive as jax device arrays and
    # transforming those would trigger on-device jax execution
    x, Wq, bq, Wk, bk, Wv, bv, Wo, bo = (
        np.asarray(a) for a in (x, Wq, bq, Wk, bk, Wv, bv, Wo, bo)
    )
    B, C, H, W = x.shape
    xf = np.ascontiguousarray(x.reshape(B, C, H * W)).astype(np.float32)
    perm = _perm_qk()
    wq_r = _fold8(Wq.T[:, perm]).astype(_F8)
    wk_r = _fold8(Wk.T[:, perm]).astype(_F8)
    wv_r = _fold8(np.ascontiguousarray(Wv.T)).astype(_F8)
    wo_r = _fold8(np.ascontiguousarray(Wo.T)).astype(_F8)
    bqk = np.concatenate(
        [
            bq[perm].reshape(4, P).T.astype(np.float32),
            bk[perm].reshape(4, P).T.astype(np.float32),
        ],
        axis=1,
    )
    bo8 = np.zeros((1, 2, DIM), _F8)
    bo8[0, 0, :] = bo.astype(_F8)
    shared = {
        "wqk8": np.ascontiguousarray(np.stack([wq_r, wk_r], axis=3)),
        "wvo8": np.ascontiguousarray(np.stack([wv_r, wo_r], axis=3)),
        "bqk": np.ascontiguousarray(bqk),
        "bv": np.asarray(bv, np.float32),
        "bo8": bo8,
        "ones8": np.ones(N, _F8),
        "ident16": np.eye(P, dtype=_BF),
    }
    in_maps = []
    for b in range(B):
        m = dict(shared)
        m["x8"] = np.ascontiguousarray(
            xf[b].reshape(2, 2, P, N).transpose(0, 2, 1, 3)
        ).astype(_F8)
        m["x16"] = xf[b].reshape(4, P, N).astype(_BF)
        in_maps.append(m)
    return in_maps


def kernel(x, Wq, bq, Wk, bk, Wv, bv, Wo, bo, _trace=False):
    from concourse.bass_utils import run_bass_kernel_spmd

    x = np.asarray(x)
    B, C, H, W = x.shape
    in_maps = _prep_maps(x, Wq, bq, Wk, bk, Wv, bv, Wo, bo)
    nc = build_nc()
    res = run_bass_kernel_spmd(nc, in_maps, core_ids=list(range(B)), trace=_trace)
    out = np.stack([res.results[b]["out"] for b in range(B)])
    out = out.reshape(B, C, H, W).astype(np.float32)
    if _trace:
        kernel.last_results = res
    return out


# revision 11
# speedup vs baseline: 1.0693x; 1.0130x over previous
"""Multi-head attention (dense_transformer) Trainium2 Bass kernel, v2.

Problem: x[8, 512, 32, 32]; per-batch 1x1-conv QKV projections, 8-head
attention over N=H*W=1024 positions (head_dim 64), output projection,
residual. Sharding: data-parallel over batch B=8 across the 8 cores -
one batch element per core, no collectives.

v2 strategy (vs the bf16 baseline): run every matmul in fp8e4 with the
DoubleRow perf mode.  A DoubleRow matmul takes lhsT [K,2,M] / rhs
[K,2,N] and computes sum_g W[:,g].T @ X[:,g] - two 128-deep contraction
steps in one instruction at 0.5 cycles per output row, i.e. 4x the
bf16 matmul throughput per unit of contracted work.  That collapses the
PE time from ~82us to ~30us and shifts the bottleneck to the softmax
exp (~65k free-elems), which is split across three engines:

  - ScalarE (Act): true exp via the activation LUT.
  - VectorE / GpSimd: Schraudolph-style fast exp - one fused
    tensor_scalar (x*log2e + bias) written as int8 IS the fp8e4 bit
    pattern of exp(x/8).  (e4m3 bits b ~= 8*(log2(v)+7), so
    b = s*log2(e) + 56 up to a mantissa correction.)

Other tricks:
  - All five "c = 512" contractions map c -> (ctpair, group, partition)
    so DoubleRow pairs come straight from SBUF layout; Wq/Wk columns are
    host-permuted so each head's Q/K lands as [32, 2, N] (d-halves in
    the group dim) on its own 32-partition band, making S^T a single
    DoubleRow matmul per output tile.
  - V is projected transposed (VT[j, head, d]) with stationary columns
    64..127 set to 1.0, so the AV matmul leaves rows 64..127 of PSUM
    holding the softmax denominator replicated 64x - normalization is
    one partition-offset tensor_tensor divide, no broadcasts.
  - Output projection accumulates Wo.T O (fp8 DoubleRow), + bo (K=1
    DoubleRow bias matmul), + x (bf16 identity matmul residual) in one
    PSUM group; the only epilogue is a PSUM->SBUF copy before DMA out.
"""

import sys

if "/opt/trn_rl_repo" not in sys.path:
    sys.path.insert(0, "/opt/trn_rl_repo")

import numpy as np
import ml_dtypes

import concourse.bass as bass
import concourse.mybir as mybir
from concourse.tile import TileContext

DIM = 512
NH = 8
HD = 64
N = 1024
P = 128
F32 = mybir.dt.float32
BF16 = mybir.dt.bfloat16
F8 = mybir.dt.float8e4
I8 = mybir.dt.int8
AOP = mybir.AluOpType
EXP = mybir.ActivationFunctionType.Exp
DR = mybir.MatmulPerfMode.DoubleRow

# Schraudolph fast-exp constants for e4m3 bits: for logits s (pre 1/8
# scale), bits = s*log2(e) + 7*8 - 0.344 (mantissa-linearity centering)
# + 0.5 (CoreSim truncates on f32->i8 write; HW rounding only shifts
# the softmax by a uniform factor that the normalization cancels).
EXP_C1 = 1.4426950408889634
EXP_C2 = 56.0 - 0.344 + 0.5


class FixedTileContext(TileContext):
    """Works around a walrus/bass snapshot mismatch: this walrus build
    accepts only one sync-wait command per instruction, but Tile's wait
    assigner happily attaches several. After scheduling, excess waits on
    any instruction are peeled off onto same-engine NOPs inserted right
    before it (same blocking semantics: the engine executes in order)."""

    MAX_WAITS = 1
    MAX_WAITS_DATA = 1
    _wsplit_ctr = 0

    def _split_sync_waits(self):
        seq_only = mybir.SEQUENCER_ONLY_OPCODES
        for fn in self.nc.m.functions:
            for blk in fn.blocks:
                insts = list(blk.instructions)
                out = []
                for inst in insts:
                    si = inst.sync_info
                    limit = (
                        self.MAX_WAITS
                        if inst.opcode in seq_only
                        else self.MAX_WAITS_DATA
                    )
                    if si is not None and len(si.on_wait) > limit:
                        waits = list(si.on_wait)
                        movers = waits[:-limit]
                        keep = waits[-limit:]
                        del si.on_wait[:]
                        for w in keep:
                            si.on_wait.append(w)
                        for w in movers:
                            FixedTileContext._wsplit_ctr += 1
                            nop = mybir.InstNoOp(
                                name=f"wsplit-{FixedTileContext._wsplit_ctr}",
                                ins=[],
                                outs=[],
                            )
                            nop.engine = inst.engine
                            nop.sync_info = mybir.SyncInfo(on_wait=[w], on_update=[])
                            out.append(nop)
                    out.append(inst)
                if len(out) != len(insts):
                    del blk.instructions[:]
                    for i in out:
                        blk.add_instruction(i)

    split_on_exit = True

    def __exit__(self, *exc):
        ret = super().__exit__(*exc)
        if exc[0] is None and self.split_on_exit:
            self._split_sync_waits()
        return ret


def build_nc(split_waits=True):
    nc = bass.Bass()

    x8d = nc.dram_tensor("x8", [2, P, 2, N], F8, kind="ExternalInput")
    x16d = nc.dram_tensor("x16", [4, P, N], BF16, kind="ExternalInput")
    wqkd = nc.dram_tensor("wqk8", [2, P, 2, 2, DIM], F8, kind="ExternalInput")
    wvod = nc.dram_tensor("wvo8", [2, P, 2, 2, DIM], F8, kind="ExternalInput")
    bqkd = nc.dram_tensor("bqk", [P, 8], F32, kind="ExternalInput")
    bvd = nc.dram_tensor("bv", [DIM], F32, kind="ExternalInput")
    bo8d = nc.dram_tensor("bo8", [1, 2, DIM], F8, kind="ExternalInput")
    onesd = nc.dram_tensor("ones8", [N], F8, kind="ExternalInput")
    identd = nc.dram_tensor("ident16", [P, P], BF16, kind="ExternalInput")
    outd = nc.dram_tensor("out", [4, P, N], BF16, kind="ExternalOutput")

    FixedTileContext.split_on_exit = split_waits
    with FixedTileContext(nc) as tc:
        with tc.tile_pool(name="persist", bufs=1) as persist, tc.tile_pool(
            name="otile", bufs=2
        ) as otile:
            # ---------------- loads ----------------
            # Startup-critical DMAs are split into halves and spread over
            # the SP and Act HWDGE queues so the first Q-projection group
            # (x8 n-half 0 + the two Wq halves) lands as early as possible.
            x8sb = [
                persist.tile([P, 2, N], F8, tag=f"x8_{cp}", name=f"x8_{cp}")
                for cp in range(2)
            ]
            wqk = [
                persist.tile([P, 2, 2, DIM], F8, tag=f"wqk_{cp}", name=f"wqk_{cp}")
                for cp in range(2)
            ]
            for cp in range(2):
                nc.sync.dma_start(
                    out=x8sb[cp][:, :, 0:512], in_=x8d[cp][:, :, 0:512]
                )
                nc.scalar.dma_start(
                    out=wqk[cp][:, :, 0, :], in_=wqkd[cp][:, :, 0, :]
                )
            for cp in range(2):
                nc.sync.dma_start(
                    out=x8sb[cp][:, :, 512:N], in_=x8d[cp][:, :, 512:N]
                )
                nc.scalar.dma_start(
                    out=wqk[cp][:, :, 1, :], in_=wqkd[cp][:, :, 1, :]
                )
            wvo = []
            for cp in range(2):
                t = persist.tile(
                    [P, 2, 2, DIM], F8, tag=f"wvo_{cp}", name=f"wvo_{cp}"
                )
                nc.sync.dma_start(out=t, in_=wvod[cp])
                wvo.append(t)
            bqk_sb = persist.tile([P, 8], F32, tag="bqk", name="bqk")
            nc.scalar.dma_start(out=bqk_sb, in_=bqkd[:, :])
            # bv broadcast across partitions and the head-group dim via
            # zero strides on the DRAM side.
            bvB = persist.tile([P, 2, NH, HD], F32, tag="bvB", name="bvB")
            bv_ap = bvd[:]
            nc.scalar.dma_start(
                out=bvB,
                in_=bass.AP(
                    tensor=bv_ap.tensor,
                    offset=0,
                    ap=[[0, P], [0, 2], [1, DIM]],
                ),
            )

            # VT tiles: [j-partition, jt-group, head, 128] where columns
            # 0..63 are V (+bv) and 64..127 are 1.0 (denominator trick).
            VT = [
                persist.tile([P, 2, NH, P], F8, tag=f"vt_{jp}", name=f"vt_{jp}")
                for jp in range(4)
            ]
            ones_ap = onesd[:]
            for jp in range(4):
                nc.sync.dma_start(
                    out=VT[jp][:, :, :, HD:P],
                    in_=bass.AP(
                        tensor=ones_ap.tensor, offset=0, ap=[[0, P], [1, N]]
                    ),
                )
            # phase-3 inputs (SP queue keeps filling while PE works)
            x16sb = []
            for ot in range(4):
                t = persist.tile([P, N], BF16, tag=f"x16_{ot}", name=f"x16_{ot}")
                nc.sync.dma_start(out=t, in_=x16d[ot])
                x16sb.append(t)
            ident16 = persist.tile([P, P], BF16, tag="ident", name="ident")
            nc.sync.dma_start(out=ident16, in_=identd[:, :])
            bo8sb = persist.tile([1, 2, DIM], F8, tag="bo8", name="bo8")
            nc.sync.dma_start(out=bo8sb, in_=bo8d[:, :, :])
            ones8 = persist.tile([1, 2, DIM], F8, tag="ones8", name="ones8")
            nc.sync.dma_start(
                out=ones8,
                in_=bass.AP(tensor=ones_ap.tensor, offset=0, ap=[[0, 1], [1, N]]),
            )

            # persistent attention state
            Qs = [
                persist.tile([P, 2, N], F8, tag=f"qs_{i}", name=f"qs_{i}")
                for i in range(2)
            ]
            Ks = [
                persist.tile([P, 2, N], F8, tag=f"ks_{i}", name=f"ks_{i}")
                for i in range(2)
            ]
            P8 = [
                [
                    persist.tile(
                        [P, 2, N], F8, tag=f"p8_{h}_{jp}", name=f"p8_{h}_{jp}"
                    )
                    for jp in range(4)
                ]
                for h in range(NH)
            ]
            O8 = [
                persist.tile([P, 2, N], F8, tag=f"o8_{cp}", name=f"o8_{cp}")
                for cp in range(2)
            ]
            stage = [
                persist.tile([HD, N], F8, tag=f"stg_{i}", name=f"stg_{i}")
                for i in range(4)
            ]

            # ------------- Q/K/V projections -------------
            with tc.tile_pool(name="pp", bufs=4, space="PSUM") as pp:
                def qk_proj(qk, ot):
                    ps = pp.tile([P, N], F32, tag="pp", name=f"pp_{qk}{ot}")
                    for nh2 in range(2):
                        for cp in range(2):
                            nc.tensor.matmul(
                                ps[:, nh2 * 512 : (nh2 + 1) * 512],
                                lhsT=wqk[cp][:, :, qk, ot * P : (ot + 1) * P],
                                rhs=x8sb[cp][:, :, nh2 * 512 : (nh2 + 1) * 512],
                                start=(cp == 0),
                                stop=(cp == 1),
                                perf_mode=DR,
                            )
                    return ps

                for ot in range(4):
                    ps = qk_proj(0, ot)
                    # Q epilogue on Act: out = Identity(ps + bq_col), fp8
                    nc.scalar.activation(
                        Qs[ot // 2][:, ot % 2, :],
                        ps,
                        mybir.ActivationFunctionType.Identity,
                        bias=bqk_sb[:, ot : ot + 1],
                    )
                for ot in range(4):
                    ps = qk_proj(1, ot)
                    # K epilogue on GpSimd (tensor_scalar, default eff)
                    nc.gpsimd.tensor_scalar(
                        Ks[ot // 2][:, ot % 2, :],
                        ps,
                        bqk_sb[:, 4 + ot : 5 + ot],
                        None,
                        op0=AOP.add,
                    )
                for vt in range(4):
                    ps = pp.tile([P, N], F32, tag="pp", name=f"pp_v{vt}")
                    for g in range(2):
                        jt = 2 * vt + g
                        for cp in range(2):
                            nc.tensor.matmul(
                                ps[:, g * 512 : (g + 1) * 512],
                                lhsT=x8sb[cp][:, :, jt * P : (jt + 1) * P],
                                rhs=wvo[cp][:, :, 0, :],
                                start=(cp == 0),
                                stop=(cp == 1),
                                perf_mode=DR,
                            )
                    # V epilogue on DVE: V^T + bv -> fp8 into VT cols 0..63
                    nc.vector.tensor_tensor(
                        VT[vt][:, :, :, 0:HD],
                        ps.rearrange("p (g h d) -> p g h d", g=2, h=NH),
                        bvB,
                        AOP.add,
                    )

            # ------------- attention heads -------------
            # Software-pipelined: AV(h-1) is emitted between S(h) jt 0..3
            # and jt 4..7, and norm(h-1) after, so the PE never blocks on
            # the exp tail of the head it just scored.
            # exp rotation: even heads Act x4 / DVE x2 / Pool x2 with the
            # normalize divide on DVE; odd heads Act x3 / DVE x3 / Pool x2
            # with the divide on Pool.
            ROT = [
                ["a", "d", "p", "a", "d", "a", "p", "a"],
                ["a", "d", "p", "a", "d", "d", "p", "a"],
            ]

            def emit_exp(eng, tgt, psS):
                if eng == "a":
                    nc.scalar.activation(tgt, psS, EXP, scale=0.125)
                elif eng == "d":
                    nc.vector.tensor_scalar(
                        tgt.bitcast(I8), psS, EXP_C1, EXP_C2,
                        op0=AOP.mult, op1=AOP.add,
                    )
                else:
                    nc.gpsimd.tensor_scalar(
                        tgt.bitcast(I8), psS, EXP_C1, EXP_C2,
                        op0=AOP.mult, op1=AOP.add,
                    )

            with tc.tile_pool(
                name="psS", bufs=2, space="PSUM"
            ) as psS_pool, tc.tile_pool(name="psO", bufs=2, space="PSUM") as psO_pool:

                def emit_s(h, jts):
                    Qt, Kt = Qs[h // 4], Ks[h // 4]
                    p0 = 32 * (h % 4)
                    for jt in jts:
                        psS = psS_pool.tile([P, N], F32, tag="psS", name="psS")
                        for ih in range(2):
                            nc.tensor.matmul(
                                psS[:, ih * 512 : (ih + 1) * 512],
                                lhsT=Kt[p0 : p0 + 32, :, jt * P : (jt + 1) * P],
                                rhs=Qt[p0 : p0 + 32, :, ih * 512 : (ih + 1) * 512],
                                start=True,
                                stop=True,
                                perf_mode=DR,
                                tile_position=(p0, 0),
                            )
                        emit_exp(ROT[h % 2][jt], P8[h][jt // 2][:, jt % 2, :], psS)

                def emit_av(h):
                    psO = psO_pool.tile([P, N], F32, tag="psO", name="psO")
                    for ih in range(2):
                        for jp in range(4):
                            nc.tensor.matmul(
                                psO[:, ih * 512 : (ih + 1) * 512],
                                lhsT=VT[jp][:, :, h, :],
                                rhs=P8[h][jp][:, :, ih * 512 : (ih + 1) * 512],
                                start=(jp == 0),
                                stop=(jp == 3),
                                perf_mode=DR,
                            )
                    return psO

                def emit_norm(h, psO):
                    # rows 64..127 of psO all hold the denominator
                    cph, gh = h // 4, (h % 4) // 2
                    div = nc.vector if h % 2 == 0 else nc.gpsimd
                    if h % 2 == 0:
                        div.tensor_tensor(
                            O8[cph][0:HD, gh, :],
                            psO[0:HD, :],
                            psO[HD:P, :],
                            AOP.divide,
                        )
                    else:
                        st = stage[h // 2]
                        div.tensor_tensor(
                            st, psO[0:HD, :], psO[HD:P, :], AOP.divide
                        )
                        nc.scalar.dma_start(out=O8[cph][HD:P, gh, :], in_=st)

                prev = None
                for h in range(NH):
                    emit_s(h, range(0, 4))
                    if prev is not None:
                        prev_ps = emit_av(prev)
                    emit_s(h, range(4, 8))
                    if prev is not None:
                        emit_norm(prev, prev_ps)
                    prev = h
                emit_norm(7, emit_av(7))

            # ------------- output projection + residual -------------
            with tc.tile_pool(name="po", bufs=4, space="PSUM") as po_pool:
                for ot in range(4):
                    po = po_pool.tile([P, N], F32, tag="po", name=f"po_{ot}")
                    for nh2 in range(2):
                        sl = slice(nh2 * 512, (nh2 + 1) * 512)
                        for cp in range(2):
                            nc.tensor.matmul(
                                po[:, sl],
                                lhsT=wvo[cp][:, :, 1, ot * P : (ot + 1) * P],
                                rhs=O8[cp][:, :, sl],
                                start=(cp == 0),
                                stop=False,
                                perf_mode=DR,
                            )
                        nc.tensor.matmul(
                            po[:, sl],
                            lhsT=bo8sb[:, :, ot * P : (ot + 1) * P],
                            rhs=ones8[:, :, :],
                            start=False,
                            stop=False,
                            perf_mode=DR,
                        )
                        nc.tensor.matmul(
                            po[:, sl],
                            lhsT=ident16,
                            rhs=x16sb[ot][:, sl],
                            start=False,
                            stop=True,
                        )
                    ob = otile.tile([P, N], BF16, tag="ob", name="ob")
                    for nh2 in range(2):
                        sl = slice(nh2 * 512, (nh2 + 1) * 512)
                        if ot % 2 == 0:
                            nc.scalar.copy(ob[:, sl], po[:, sl])
                            nc.scalar.dma_start(out=outd[ot][:, sl], in_=ob[:, sl])
                        else:
                            nc.vector.tensor_copy(ob[:, sl], po[:, sl])
                            nc.sync.dma_start(out=outd[ot][:, sl], in_=ob[:, sl])
    return nc


_BF = ml_dtypes.bfloat16
_F8 = ml_dtypes.float8_e4m3


def _perm_qk():
    # PSUM partition p of Q/K projection tile `ot` holds output row
    # o = (4*(ot//2) + p//32)*64 + 32*(ot%2) + p%32  (head-banded,
    # d-halves split across the DoubleRow group dim).
    j = np.arange(DIM)
    ot, pp = j // P, j % P
    return (4 * (ot // 2) + pp // 32) * HD + 32 * (ot % 2) + (pp % 32)


def _fold8(a):
    # [512, M] c-major -> [ctpair, partition, group, M]
    M = a.shape[1]
    return np.ascontiguousarray(
        a.reshape(2, 2, P, M).transpose(0, 2, 1, 3)
    )


def _prep_maps(x, Wq, bq, Wk, bk, Wv, bv, Wo, bo):
    # plain numpy up front: inputs may arrive as jax device arrays and
    # transforming those would trigger on-device jax execution
    x, Wq, bq, Wk, bk, Wv, bv, Wo, bo = (
        np.asarray(a) for a in (x, Wq, bq, Wk, bk, Wv, bv, Wo, bo)
    )
    B, C, H, W = x.shape
    xf = np.ascontiguousarray(x.reshape(B, C, H * W)).astype(np.float32)
    perm = _perm_qk()
    wq_r = _fold8(Wq.T[:, perm]).astype(_F8)
    wk_r = _fold8(Wk.T[:, perm]).astype(_F8)
    wv_r = _fold8(np.ascontiguousarray(Wv.T)).astype(_F8)
    wo_r = _fold8(np.ascontiguousarray(Wo.T)).astype(_F8)
    bqk = np.concatenate(
        [
            bq[perm].reshape(4, P).T.astype(np.float32),
            bk[perm].reshape(4, P).T.astype(np.float32),
        ],
        axis=1,
    )
    bo8 = np.zeros((1, 2, DIM), _F8)
    bo8[0, 0, :] = bo.astype(_F8)
    shared = {
        "wqk8": np.ascontiguousarray(np.stack([wq_r, wk_r], axis=3)),
        "wvo8": np.ascontiguousarray(np.stack([wv_r, wo_r], axis=3)),
        "bqk": np.ascontiguousarray(bqk),
        "bv": np.asarray(bv, np.float32),
        "bo8": bo8,
        "ones8": np.ones(N, _F8),
        "ident16": np.eye(P, dtype=_BF),
    }
    in_maps = []
    for b in range(B):
        m = dict(shared)
        m["x8"] = np.ascontiguousarray(
            xf[b].reshape(2, 2, P, N).transpose(0, 2, 1, 3)
        ).astype(_F8)
        m["x16"] = xf[b].reshape(4, P, N).astype(_BF)
        in_maps.append(m)
    return in_maps


def kernel(x, Wq, bq, Wk, bk, Wv, bv, Wo, bo, _trace=False):
    from concourse.bass_utils import run_bass_kernel_spmd

    x = np.asarray(x)
    B, C, H, W = x.shape
    in_maps = _prep_maps(x, Wq, bq, Wk, bk, Wv, bv, Wo, bo)
    nc = build_nc()
    res = run_bass_kernel_spmd(nc, in_maps, core_ids=list(range(B)), trace=_trace)
    out = np.stack(
        [np.asarray(res.results[b]["out"]).astype(np.float32) for b in range(B)]
    )
    out = out.reshape(B, C, H, W)
    if _trace:
        kernel.last_results = res
    return out


# revision 12
# speedup vs baseline: 1.2991x; 1.2149x over previous
"""Multi-head attention (dense_transformer) Trainium2 Bass kernel, v2.

Problem: x[8, 512, 32, 32]; per-batch 1x1-conv QKV projections, 8-head
attention over N=H*W=1024 positions (head_dim 64), output projection,
residual. Sharding: data-parallel over batch B=8 across the 8 cores -
one batch element per core, no collectives.

v2 strategy (vs the bf16 baseline): run every matmul in fp8e4 with the
DoubleRow perf mode.  A DoubleRow matmul takes lhsT [K,2,M] / rhs
[K,2,N] and computes sum_g W[:,g].T @ X[:,g] - two 128-deep contraction
steps in one instruction at 0.5 cycles per output row, i.e. 4x the
bf16 matmul throughput per unit of contracted work.  That collapses the
PE time from ~82us to ~30us and shifts the bottleneck to the softmax
exp (~65k free-elems), which is split across three engines:

  - ScalarE (Act): true exp via the activation LUT.
  - VectorE / GpSimd: Schraudolph-style fast exp - one fused
    tensor_scalar (x*log2e + bias) written as int8 IS the fp8e4 bit
    pattern of exp(x/8).  (e4m3 bits b ~= 8*(log2(v)+7), so
    b = s*log2(e) + 56 up to a mantissa correction.)

Other tricks:
  - All five "c = 512" contractions map c -> (ctpair, group, partition)
    so DoubleRow pairs come straight from SBUF layout; Wq/Wk columns are
    host-permuted so each head's Q/K lands as [32, 2, N] (d-halves in
    the group dim) on its own 32-partition band, making S^T a single
    DoubleRow matmul per output tile.
  - V is projected transposed (VT[j, head, d]) with stationary columns
    64..127 set to 1.0, so the AV matmul leaves rows 64..127 of PSUM
    holding the softmax denominator replicated 64x - normalization is
    one partition-offset tensor_tensor divide, no broadcasts.
  - Output projection accumulates Wo.T O (fp8 DoubleRow), + bo (K=1
    DoubleRow bias matmul), + x (bf16 identity matmul residual) in one
    PSUM group; the only epilogue is a PSUM->SBUF copy before DMA out.
"""

import sys

if "/opt/trn_rl_repo" not in sys.path:
    sys.path.insert(0, "/opt/trn_rl_repo")

import numpy as np
import ml_dtypes

import concourse.bass as bass
import concourse.mybir as mybir
from concourse.tile import TileContext

DIM = 512
NH = 8
HD = 64
N = 1024
P = 128
F32 = mybir.dt.float32
BF16 = mybir.dt.bfloat16
F8 = mybir.dt.float8e4
I8 = mybir.dt.int8
AOP = mybir.AluOpType
EXP = mybir.ActivationFunctionType.Exp
DR = mybir.MatmulPerfMode.DoubleRow

# Schraudolph fast-exp constants for e4m3 bits: for logits s (pre 1/8
# scale), bits = s*log2(e) + 7*8 - 0.344 (mantissa-linearity centering)
# + 0.5 (CoreSim truncates on f32->i8 write; HW rounding only shifts
# the softmax by a uniform factor that the normalization cancels).
EXP_C1 = 1.4426950408889634
EXP_C2 = 56.0 - 0.344 + 0.5


class FixedTileContext(TileContext):
    """Works around a walrus/bass snapshot mismatch: this walrus build
    accepts only one sync-wait command per instruction, but Tile's wait
    assigner happily attaches several. After scheduling, excess waits on
    any instruction are peeled off onto same-engine NOPs inserted right
    before it (same blocking semantics: the engine executes in order)."""

    MAX_WAITS = 1
    MAX_WAITS_DATA = 1
    _wsplit_ctr = 0

    def _split_sync_waits(self):
        seq_only = mybir.SEQUENCER_ONLY_OPCODES
        for fn in self.nc.m.functions:
            for blk in fn.blocks:
                insts = list(blk.instructions)
                out = []
                for inst in insts:
                    si = inst.sync_info
                    limit = (
                        self.MAX_WAITS
                        if inst.opcode in seq_only
                        else self.MAX_WAITS_DATA
                    )
                    if si is not None and len(si.on_wait) > limit:
                        waits = list(si.on_wait)
                        movers = waits[:-limit]
                        keep = waits[-limit:]
                        del si.on_wait[:]
                        for w in keep:
                            si.on_wait.append(w)
                        for w in movers:
                            FixedTileContext._wsplit_ctr += 1
                            nop = mybir.InstNoOp(
                                name=f"wsplit-{FixedTileContext._wsplit_ctr}",
                                ins=[],
                                outs=[],
                            )
                            nop.engine = inst.engine
                            nop.sync_info = mybir.SyncInfo(on_wait=[w], on_update=[])
                            out.append(nop)
                    out.append(inst)
                if len(out) != len(insts):
                    del blk.instructions[:]
                    for i in out:
                        blk.add_instruction(i)

    split_on_exit = True

    def __exit__(self, *exc):
        ret = super().__exit__(*exc)
        if exc[0] is None and self.split_on_exit:
            self._split_sync_waits()
        return ret


def build_nc(split_waits=True):
    nc = bass.Bass()

    x8d = nc.dram_tensor("x8", [2, P, 2, N], F8, kind="ExternalInput")
    x16d = nc.dram_tensor("x16", [4, P, N], BF16, kind="ExternalInput")
    wqkd = nc.dram_tensor("wqk8", [2, P, 2, 2, DIM], F8, kind="ExternalInput")
    wvod = nc.dram_tensor("wvo8", [2, P, 2, 2, DIM], F8, kind="ExternalInput")
    bqkd = nc.dram_tensor("bqk", [P, 8], F32, kind="ExternalInput")
    bvd = nc.dram_tensor("bv", [DIM], F32, kind="ExternalInput")
    bo8d = nc.dram_tensor("bo8", [1, 2, DIM], F8, kind="ExternalInput")
    onesd = nc.dram_tensor("ones8", [N], F8, kind="ExternalInput")
    identd = nc.dram_tensor("ident16", [P, P], BF16, kind="ExternalInput")
    outd = nc.dram_tensor("out", [4, P, N], BF16, kind="ExternalOutput")

    FixedTileContext.split_on_exit = split_waits
    with FixedTileContext(nc) as tc:
        with tc.tile_pool(name="persist", bufs=1) as persist, tc.tile_pool(
            name="otile", bufs=2
        ) as otile:
            # ---------------- loads ----------------
            # Startup-critical DMAs are split into halves and spread over
            # the SP and Act HWDGE queues so the first Q-projection group
            # (x8 n-half 0 + the two Wq halves) lands as early as possible.
            x8sb = [
                persist.tile([P, 2, N], F8, tag=f"x8_{cp}", name=f"x8_{cp}")
                for cp in range(2)
            ]
            wqk = [
                persist.tile([P, 2, 2, DIM], F8, tag=f"wqk_{cp}", name=f"wqk_{cp}")
                for cp in range(2)
            ]
            for cp in range(2):
                nc.sync.dma_start(
                    out=x8sb[cp][:, :, 0:512], in_=x8d[cp][:, :, 0:512]
                )
                nc.scalar.dma_start(
                    out=wqk[cp][:, :, 0, :], in_=wqkd[cp][:, :, 0, :]
                )
            for cp in range(2):
                nc.sync.dma_start(
                    out=x8sb[cp][:, :, 512:N], in_=x8d[cp][:, :, 512:N]
                )
                nc.scalar.dma_start(
                    out=wqk[cp][:, :, 1, :], in_=wqkd[cp][:, :, 1, :]
                )
            wvo = []
            for cp in range(2):
                t = persist.tile(
                    [P, 2, 2, DIM], F8, tag=f"wvo_{cp}", name=f"wvo_{cp}"
                )
                nc.sync.dma_start(out=t, in_=wvod[cp])
                wvo.append(t)
            bqk_sb = persist.tile([P, 8], F32, tag="bqk", name="bqk")
            nc.scalar.dma_start(out=bqk_sb, in_=bqkd[:, :])
            # bv broadcast across partitions and the head-group dim via
            # zero strides on the DRAM side.
            bvB = persist.tile([P, 2, NH, HD], F32, tag="bvB", name="bvB")
            bv_ap = bvd[:]
            nc.scalar.dma_start(
                out=bvB,
                in_=bass.AP(
                    tensor=bv_ap.tensor,
                    offset=0,
                    ap=[[0, P], [0, 2], [1, DIM]],
                ),
            )

            # VT tiles: [j-partition, jt-group, head, 128] where columns
            # 0..63 are V (+bv) and 64..127 are 1.0 (denominator trick).
            VT = [
                persist.tile([P, 2, NH, P], F8, tag=f"vt_{jp}", name=f"vt_{jp}")
                for jp in range(4)
            ]
            ones_ap = onesd[:]
            for jp in range(4):
                nc.sync.dma_start(
                    out=VT[jp][:, :, :, HD:P],
                    in_=bass.AP(
                        tensor=ones_ap.tensor, offset=0, ap=[[0, P], [1, N]]
                    ),
                )
            # phase-3 inputs (SP queue keeps filling while PE works)
            x16sb = []
            for ot in range(4):
                t = persist.tile([P, N], BF16, tag=f"x16_{ot}", name=f"x16_{ot}")
                nc.sync.dma_start(out=t, in_=x16d[ot])
                x16sb.append(t)
            ident16 = persist.tile([P, P], BF16, tag="ident", name="ident")
            nc.sync.dma_start(out=ident16, in_=identd[:, :])
            bo8sb = persist.tile([1, 2, DIM], F8, tag="bo8", name="bo8")
            nc.sync.dma_start(out=bo8sb, in_=bo8d[:, :, :])
            ones8 = persist.tile([1, 2, DIM], F8, tag="ones8", name="ones8")
            nc.sync.dma_start(
                out=ones8,
                in_=bass.AP(tensor=ones_ap.tensor, offset=0, ap=[[0, 1], [1, N]]),
            )

            # persistent attention state
            Qs = [
                persist.tile([P, 2, N], F8, tag=f"qs_{i}", name=f"qs_{i}")
                for i in range(2)
            ]
            Ks = [
                persist.tile([P, 2, N], F8, tag=f"ks_{i}", name=f"ks_{i}")
                for i in range(2)
            ]
            P8 = [
                [
                    persist.tile(
                        [P, 2, N], F8, tag=f"p8_{h}_{jp}", name=f"p8_{h}_{jp}"
                    )
                    for jp in range(4)
                ]
                for h in range(NH)
            ]
            O8 = [
                persist.tile([P, 2, N], F8, tag=f"o8_{cp}", name=f"o8_{cp}")
                for cp in range(2)
            ]
            stage = [
                persist.tile([HD, N], F8, tag=f"stg_{i}", name=f"stg_{i}")
                for i in range(4)
            ]

            # ------------- Q/K/V projections -------------
            with tc.tile_pool(name="pp", bufs=4, space="PSUM") as pp:
                def qk_proj(qk, ot):
                    ps = pp.tile([P, N], F32, tag="pp", name=f"pp_{qk}{ot}")
                    for nh2 in range(2):
                        for cp in range(2):
                            nc.tensor.matmul(
                                ps[:, nh2 * 512 : (nh2 + 1) * 512],
                                lhsT=wqk[cp][:, :, qk, ot * P : (ot + 1) * P],
                                rhs=x8sb[cp][:, :, nh2 * 512 : (nh2 + 1) * 512],
                                start=(cp == 0),
                                stop=(cp == 1),
                                perf_mode=DR,
                            )
                    return ps

                for ot in range(4):
                    ps = qk_proj(0, ot)
                    # Q epilogue on Act: out = Identity(ps + bq_col), fp8
                    nc.scalar.activation(
                        Qs[ot // 2][:, ot % 2, :],
                        ps,
                        mybir.ActivationFunctionType.Identity,
                        bias=bqk_sb[:, ot : ot + 1],
                    )
                for ot in range(4):
                    ps = qk_proj(1, ot)
                    # K epilogue on GpSimd (tensor_scalar, default eff)
                    nc.gpsimd.tensor_scalar(
                        Ks[ot // 2][:, ot % 2, :],
                        ps,
                        bqk_sb[:, 4 + ot : 5 + ot],
                        None,
                        op0=AOP.add,
                    )
                for vt in range(4):
                    ps = pp.tile([P, N], F32, tag="pp", name=f"pp_v{vt}")
                    for g in range(2):
                        jt = 2 * vt + g
                        for cp in range(2):
                            nc.tensor.matmul(
                                ps[:, g * 512 : (g + 1) * 512],
                                lhsT=x8sb[cp][:, :, jt * P : (jt + 1) * P],
                                rhs=wvo[cp][:, :, 0, :],
                                start=(cp == 0),
                                stop=(cp == 1),
                                perf_mode=DR,
                            )
                    # V epilogue on DVE: V^T + bv -> fp8 into VT cols 0..63
                    nc.vector.tensor_tensor(
                        VT[vt][:, :, :, 0:HD],
                        ps.rearrange("p (g h d) -> p g h d", g=2, h=NH),
                        bvB,
                        AOP.add,
                    )

            # ------------- attention heads -------------
            # Software-pipelined: AV(h-1) is emitted between S(h) jt 0..3
            # and jt 4..7, and norm(h-1) after, so the PE never blocks on
            # the exp tail of the head it just scored.
            # exp rotation: even heads Act x4 / DVE x2 / Pool x2 with the
            # normalize divide on DVE; odd heads Act x3 / DVE x3 / Pool x2
            # with the divide on Pool.
            ROT = [
                ["a", "d", "p", "a", "d", "a", "p", "a"],
                ["a", "d", "p", "a", "d", "d", "p", "a"],
            ]

            def emit_exp(eng, tgt, psS):
                if eng == "a":
                    nc.scalar.activation(tgt, psS, EXP, scale=0.125)
                elif eng == "d":
                    nc.vector.tensor_scalar(
                        tgt.bitcast(I8), psS, EXP_C1, EXP_C2,
                        op0=AOP.mult, op1=AOP.add,
                    )
                else:
                    nc.gpsimd.tensor_scalar(
                        tgt.bitcast(I8), psS, EXP_C1, EXP_C2,
                        op0=AOP.mult, op1=AOP.add,
                    )

            with tc.tile_pool(
                name="psS", bufs=3, space="PSUM"
            ) as psS_pool, tc.tile_pool(name="psO", bufs=1, space="PSUM") as psO_pool:

                def emit_s(h, jts):
                    Qt, Kt = Qs[h // 4], Ks[h // 4]
                    p0 = 32 * (h % 4)
                    for jt in jts:
                        psS = psS_pool.tile([P, N], F32, tag="psS", name="psS")
                        for ih in range(2):
                            nc.tensor.matmul(
                                psS[:, ih * 512 : (ih + 1) * 512],
                                lhsT=Kt[p0 : p0 + 32, :, jt * P : (jt + 1) * P],
                                rhs=Qt[p0 : p0 + 32, :, ih * 512 : (ih + 1) * 512],
                                start=True,
                                stop=True,
                                perf_mode=DR,
                                tile_position=(p0, 0),
                            )
                        emit_exp(ROT[h % 2][jt], P8[h][jt // 2][:, jt % 2, :], psS)

                def emit_av(h):
                    psO = psO_pool.tile([P, N], F32, tag="psO", name="psO")
                    for ih in range(2):
                        for jp in range(4):
                            nc.tensor.matmul(
                                psO[:, ih * 512 : (ih + 1) * 512],
                                lhsT=VT[jp][:, :, h, :],
                                rhs=P8[h][jp][:, :, ih * 512 : (ih + 1) * 512],
                                start=(jp == 0),
                                stop=(jp == 3),
                                perf_mode=DR,
                            )
                    return psO

                def emit_norm(h, psO):
                    # rows 64..127 of psO all hold the denominator
                    cph, gh = h // 4, (h % 4) // 2
                    div = nc.vector if h % 2 == 0 else nc.gpsimd
                    if h % 2 == 0:
                        div.tensor_tensor(
                            O8[cph][0:HD, gh, :],
                            psO[0:HD, :],
                            psO[HD:P, :],
                            AOP.divide,
                        )
                    else:
                        st = stage[h // 2]
                        div.tensor_tensor(
                            st, psO[0:HD, :], psO[HD:P, :], AOP.divide
                        )
                        nc.scalar.dma_start(out=O8[cph][HD:P, gh, :], in_=st)

                prev = None
                for h in range(NH):
                    emit_s(h, range(0, 4))
                    if prev is not None:
                        prev_ps = emit_av(prev)
                    emit_s(h, range(4, 8))
                    if prev is not None:
                        emit_norm(prev, prev_ps)
                    prev = h
                emit_norm(7, emit_av(7))

            # ------------- output projection + residual -------------
            with tc.tile_pool(name="po", bufs=4, space="PSUM") as po_pool:
                for ot in range(4):
                    po = po_pool.tile([P, N], F32, tag="po", name=f"po_{ot}")
                    for nh2 in range(2):
                        sl = slice(nh2 * 512, (nh2 + 1) * 512)
                        for cp in range(2):
                            nc.tensor.matmul(
                                po[:, sl],
                                lhsT=wvo[cp][:, :, 1, ot * P : (ot + 1) * P],
                                rhs=O8[cp][:, :, sl],
                                start=(cp == 0),
                                stop=False,
                                perf_mode=DR,
                            )
                        nc.tensor.matmul(
                            po[:, sl],
                            lhsT=bo8sb[:, :, ot * P : (ot + 1) * P],
                            rhs=ones8[:, :, :],
                            start=False,
                            stop=False,
                            perf_mode=DR,
                        )
                        nc.tensor.matmul(
                            po[:, sl],
                            lhsT=ident16,
                            rhs=x16sb[ot][:, sl],
                            start=False,
                            stop=True,
                        )
                    ob = otile.tile([P, N], BF16, tag="ob", name="ob")
                    for nh2 in range(2):
                        sl = slice(nh2 * 512, (nh2 + 1) * 512)
                        if ot % 2 == 0:
                            nc.scalar.copy(ob[:, sl], po[:, sl])
                            nc.scalar.dma_start(out=outd[ot][:, sl], in_=ob[:, sl])
                        else:
                            nc.vector.tensor_copy(ob[:, sl], po[:, sl])
                            nc.sync.dma_start(out=outd[ot][:, sl], in_=ob[:, sl])
    return nc


_BF = ml_dtypes.bfloat16
_F8 = ml_dtypes.float8_e4m3


def _perm_qk():
    # PSUM partition p of Q/K projection tile `ot` holds output row
    # o = (4*(ot//2) + p//32)*64 + 32*(ot%2) + p%32  (head-banded,
    # d-halves split across the DoubleRow group dim).
    j = np.arange(DIM)
    ot, pp = j // P, j % P
    return (4 * (ot // 2) + pp // 32) * HD + 32 * (ot % 2) + (pp % 32)


def _fold8(a):
    # [512, M] c-major -> [ctpair, partition, group, M]
    M = a.shape[1]
    return np.ascontiguousarray(
        a.reshape(2, 2, P, M).transpose(0, 2, 1, 3)
    )


def _prep_maps(x, Wq, bq, Wk, bk, Wv, bv, Wo, bo):
    # plain numpy up front: inputs may arrive as jax device arrays and
    # transforming those would trigger on-device jax execution
    x, Wq, bq, Wk, bk, Wv, bv, Wo, bo = (
        np.asarray(a) for a in (x, Wq, bq, Wk, bk, Wv, bv, Wo, bo)
    )
    B, C, H, W = x.shape
    xf = np.ascontiguousarray(x.reshape(B, C, H * W)).astype(np.float32)
    perm = _perm_qk()
    wq_r = _fold8(Wq.T[:, perm]).astype(_F8)
    wk_r = _fold8(Wk.T[:, perm]).astype(_F8)
    wv_r = _fold8(np.ascontiguousarray(Wv.T)).astype(_F8)
    wo_r = _fold8(np.ascontiguousarray(Wo.T)).astype(_F8)
    bqk = np.concatenate(
        [
            bq[perm].reshape(4, P).T.astype(np.float32),
            bk[perm].reshape(4, P).T.astype(np.float32),
        ],
        axis=1,
    )
    bo8 = np.zeros((1, 2, DIM), _F8)
    bo8[0, 0, :] = bo.astype(_F8)
    shared = {
        "wqk8": np.ascontiguousarray(np.stack([wq_r, wk_r], axis=3)),
        "wvo8": np.ascontiguousarray(np.stack([wv_r, wo_r], axis=3)),
        "bqk": np.ascontiguousarray(bqk),
        "bv": np.asarray(bv, np.float32),
        "bo8": bo8,
        "ones8": np.ones(N, _F8),
        "ident16": np.eye(P, dtype=_BF),
    }
    in_maps = []
    for b in range(B):
        m = dict(shared)
        m["x8"] = np.ascontiguousarray(
            xf[b].reshape(2, 2, P, N).transpose(0, 2, 1, 3)
        ).astype(_F8)
        m["x16"] = xf[b].reshape(4, P, N).astype(_BF)
        in_maps.append(m)
    return in_maps


def kernel(x, Wq, bq, Wk, bk, Wv, bv, Wo, bo, _trace=False):
    from concourse.bass_utils import run_bass_kernel_spmd

    x = np.asarray(x)
    B, C, H, W = x.shape
    in_maps = _prep_maps(x, Wq, bq, Wk, bk, Wv, bv, Wo, bo)
    nc = build_nc()
    res = run_bass_kernel_spmd(nc, in_maps, core_ids=list(range(B)), trace=_trace)
    out = np.stack(
        [np.asarray(res.results[b]["out"]).astype(np.float32) for b in range(B)]
    )
    out = out.reshape(B, C, H, W)
    if _trace:
        kernel.last_results = res
    return out
